# revision 1
# baseline (speedup 1.0000x reference)
"""AV temporal gated-conv MLP block for Trainium2 (8 NeuronCores, Bass/Tile).

Per-core strategy: pure data parallelism over the batch (B=8 -> 1 batch
element per core, both modalities on the same core since the gating couples
them). No collectives. Per core, loop over 4 l-blocks of 512 tokens:

  T: rms-norm in natural [l, d] layout -- ACT Square with accum_out for the
     square-sums (into one [128,4] stats tile per mod-block), rsqrt as a
     single BATCHED [128,4] DVE Newton chain (bit-trick seed + 3 iters; the
     chain's latency paces xT availability, so batching 4 l-tiles cuts the
     critical path 4x), per-partition scale, then DMA-XBAR transpose
     (dma_start(transpose=True)): one instruction turns xn [128(l),1024(d)]
     into the [128(d%128), 8(dc), 128(l)] xT slab -- ZERO PE involvement.
  A: in_proj as bf16 matmuls (1 cycle/row PE rate) producing [e, l] chunks;
     causal depthwise conv on the x-half as 1 tensor_scalar + 3
     scalar_tensor_tensor fused MACs on VectorE in BF16 (2x DVE rate); the
     3-col conv-history prepend/save copies run on ACT (tiny DVE ops pay
     ~+124ns/instr on real HW -- measured via microprobe); silu (ACT, bf16
     out) straight out of PSUM; cross-modal gate writes bf16 gat.
  B: out_proj bf16 matmuls against SBUF-RESIDENT out_proj weights (loaded
     once, 8MB bf16 both modalities, killing the 64MB f32 wout restream);
     residual add reads the retained natural-layout x tiles (no re-read).

The whole datapath is bf16 (tolerance 2e-2; measured rel-err 3.7e-3):
weights, x, xT, gat, y. Per-core HBM traffic drops 252MB (f32r) -> 88MB.

HW microprobes (slope-timed, this axon setup): pure PE matmul stream runs
at exactly the cost-model rate (ratio 0.992, 2.4GHz, Ldweights hidden);
DMA runs ~1.25x FASTER than model; DVE pays +124ns/instr over model; and
a PE stream with CONCURRENT DMA streaming runs 1.17x slower than model
(SBUF/fabric contention) -- which is why cutting DMA bytes and DVE
instruction count dominated the tuning. Cost model: 654us/rep marginal
(= the PE floor); HW slope-measured ~817us/exec (vs 872us baseline).
"""
import sys

if "/opt/trn_rl_repo" not in sys.path:
    sys.path.insert(0, "/opt/trn_rl_repo")

import numpy as np

DIM = 1024
INNER = 2048
L = 2048
B = 8
NCORES = 8
EPS = 1e-5
LB = 512              # l-block (tokens per block)
NB = L // LB          # 4 blocks
NXC = INNER // 128    # 16 x-half e-chunks per modality
NKD = DIM // 128      # 8 contraction chunks for in_proj
NLT = LB // 128       # 4 l-tiles per block
NN = DIM // 512       # 2 out_proj n-tiles
CONV_MODE = "dve"  # "dve" | "gps" | "dve+gps"  (gps fails in neuronxcc)

_cache = {}


def _build_nc(conv_mode=None, repeat=1):
    conv_mode = conv_mode or CONV_MODE
    from contextlib import ExitStack

    import concourse.bass as bass
    import concourse.tile as tile
    from concourse import bacc, mybir
    from concourse.masks import make_identity

    dt = mybir.dt
    f32 = dt.float32
    bf16 = dt.bfloat16
    i32 = dt.int32
    AOP = mybir.AluOpType
    AF = mybir.ActivationFunctionType

    nc = bacc.Bacc("TRN2", target_bir_lowering=False, debug=False,
                   num_devices=NCORES)

    x_dram = {
        "a": nc.dram_tensor("xa", [L, DIM], bf16, kind="ExternalInput").ap(),
        "v": nc.dram_tensor("xv", [L, DIM], bf16, kind="ExternalInput").ap(),
    }
    win_dram = {
        "a": nc.dram_tensor("wina", [2 * NXC, 128, NKD, 128], bf16,
                            kind="ExternalInput").ap(),
        "v": nc.dram_tensor("winv", [2 * NXC, 128, NKD, 128], bf16,
                            kind="ExternalInput").ap(),
    }
    wout_dram = {
        "a": nc.dram_tensor("wouta", [128, NXC * NN * 512], bf16,
                            kind="ExternalInput").ap(),
        "v": nc.dram_tensor("woutv", [128, NXC * NN * 512], bf16,
                            kind="ExternalInput").ap(),
    }
    cw_dram = {
        "a": nc.dram_tensor("cwa", [128, NXC * 4], f32, kind="ExternalInput").ap(),
        "v": nc.dram_tensor("cwv", [128, NXC * 4], f32, kind="ExternalInput").ap(),
    }
    cbc_dram = {
        "a": nc.dram_tensor("cbca", [128, NXC], f32, kind="ExternalInput").ap(),
        "v": nc.dram_tensor("cbcv", [128, NXC], f32, kind="ExternalInput").ap(),
    }
    y = nc.dram_tensor("y", [2 * L, DIM], bf16, kind="ExternalOutput").ap()

    MODS = ("a", "v")

    with tile.TileContext(nc) as tc, ExitStack() as ctx:
        sing = ctx.enter_context(tc.tile_pool(name="sing", bufs=1))
        p_xT = ctx.enter_context(tc.tile_pool(name="xT", bufs=2))
        p_gat = ctx.enter_context(tc.tile_pool(name="gat", bufs=2))
        p_xin = ctx.enter_context(tc.tile_pool(name="xin", bufs=18))
        p_xn = ctx.enter_context(tc.tile_pool(name="xn", bufs=6))
        p_stat = ctx.enter_context(tc.tile_pool(name="stat", bufs=4))
        p_win = ctx.enter_context(tc.tile_pool(name="win", bufs=6))
        p_axp = ctx.enter_context(tc.tile_pool(name="axp", bufs=4))
        p_sv = ctx.enter_context(tc.tile_pool(name="sv", bufs=3))
        p_yout = ctx.enter_context(tc.tile_pool(name="yout", bufs=4))
        p_ps = ctx.enter_context(
            tc.tile_pool(name="ps", bufs=8, space=bass.MemorySpace.PSUM))

        magic = sing.tile([128, NLT], i32, name="magic", tag="magic")
        nc.vector.memset(magic[:], 0x5F3759DF)

        cw_sb, cbc_sb, hist, wout_sb = {}, {}, {}, {}

        def setup_conv_state():
            for mod in MODS:
                cw_sb[mod] = sing.tile([128, NXC * 4], f32, name=f"cw_{mod}",
                                       tag=f"cw_{mod}")
                nc.sync.dma_start(cw_sb[mod][:], cw_dram[mod][:])
                cbc_sb[mod] = sing.tile([128, NXC], f32, name=f"cbc_{mod}",
                                        tag=f"cbc_{mod}")
                nc.sync.dma_start(cbc_sb[mod][:], cbc_dram[mod][:])
                hist[mod] = sing.tile([128, NXC * 3], bf16, name=f"hist_{mod}",
                                      tag=f"hist_{mod}")
                nc.vector.memset(hist[mod][:], 0.0)

        def load_wout():
            # split into 4 chunks per modality so no single 4MB DMA blocks a
            # later win-stream load that lands behind it in the same queue
            NSP = 4
            csz = NXC * NN * 512 // NSP
            for mod in MODS:
                wout_sb[mod] = sing.tile([128, NXC * NN * 512], bf16,
                                         name=f"wout_{mod}", tag=f"wout_{mod}")
                for s in range(NSP):
                    nc.sync.dma_start(wout_sb[mod][:, s * csz:(s + 1) * csz],
                                      wout_dram[mod][:, s * csz:(s + 1) * csz])

        def emit_T(blk, first=False, after_mod=None):
            l0 = blk * LB
            xTt, xts = {}, {}
            # schedule the load+square chain ~a half block early (priority-only:
            # the PE transposes keep their natural slot so PSUM isn't grabbed).
            # First block: strictly stagger mod-a chain > a-weights > mod-v
            # chain > v-weights so the scheduler doesn't round-robin the DMAs
            # (mod-a's chain latency gates the very first matmul).
            for mod in MODS:
                boost = 0 if first else 600
                if after_mod is not None and mod != MODS[0]:
                    after_mod(MODS[0])
                xTt[mod] = p_xT.tile([128, NKD * LB], bf16, name="xT", tag="xT")
                xts[mod] = []
                xns = []
                stats = p_stat.tile([128, NLT], f32, name="ssum", tag="ssum")
                with tc.high_priority(offset=boost):
                    for lt in range(NLT):
                        xt = p_xin.tile([128, DIM], bf16, name="xin", tag="xin")
                        nc.sync.dma_start(
                            xt[:], x_dram[mod][l0 + lt * 128: l0 + (lt + 1) * 128, :])
                        xnt = p_xn.tile([128, DIM], bf16, name="xn", tag="xn")
                        nc.scalar.activation(xnt[:], xt[:], AF.Square,
                                             accum_out=stats[:, lt:lt + 1])
                        xts[mod].append(xt)
                        xns.append(xnt)
                with tc.high_priority(offset=boost):
                    # rsqrt of all NLT square-sums in ONE batched [128, NLT]
                    # Newton chain (bit-trick seed + 3 iterations): 4x fewer
                    # serial DVE ops than per-l-tile chains -- this chain's
                    # latency paces xT availability and hence PE start
                    m = p_stat.tile([128, NLT], f32, name="mvar", tag="mvar")
                    nc.vector.tensor_scalar(m[:], stats[:], 1.0 / DIM, EPS,
                                            AOP.mult, AOP.add)
                    ts = p_stat.tile([128, NLT], f32, name="nsh", tag="nsh")
                    nc.vector.tensor_scalar(ts[:].bitcast(i32), m[:].bitcast(i32),
                                            1, None, AOP.logical_shift_right)
                    yv = p_stat.tile([128, NLT], f32, name="ny0", tag="ny0")
                    nc.vector.tensor_tensor(yv[:].bitcast(i32), magic[:],
                                            ts[:].bitcast(i32), AOP.subtract)
                    mh = p_stat.tile([128, NLT], f32, name="nmh", tag="nmh")
                    nc.vector.tensor_scalar(mh[:], m[:], -0.5, None, AOP.mult)
                    for it in range(3):
                        aa = p_stat.tile([128, NLT], f32, name="na", tag="na")
                        nc.vector.tensor_mul(aa[:], yv[:], yv[:])
                        cc = p_stat.tile([128, NLT], f32, name="ncc", tag="ncc")
                        nc.vector.tensor_tensor(cc[:], aa[:], mh[:], AOP.mult)
                        nc.vector.tensor_scalar(cc[:], cc[:], 1.0, 1.5,
                                                AOP.mult, AOP.add)
                        y2 = p_stat.tile([128, NLT], f32, name="nyi", tag="nyi")
                        nc.vector.tensor_mul(y2[:], yv[:], cc[:])
                        yv = y2
                    for lt in range(NLT):
                        nc.vector.tensor_scalar(xns[lt][:], xts[mod][lt][:],
                                                yv[:, lt:lt + 1], None, AOP.mult)
                with tc.high_priority(offset=boost):
                    for lt in range(NLT):
                        # DMA XBAR transpose: xn [128(l), 1024(d)] -> xT slab
                        # [128(d%128), 8(d//128), 128(l)] in one instruction
                        # (64 16x128 xbar tiles, ~0.9us) -- zero PE involvement
                        dst = xTt[mod].rearrange("p (dc l) -> p dc l", dc=NKD)[
                            :, :, lt * 128:(lt + 1) * 128]
                        nc.sync.dma_start(dst, xns[lt][:], transpose=True)
            if after_mod is not None:
                after_mod(MODS[1])

            return xTt, xts

        def prefetch_w(blk):
            tiles = {}
            for mod in MODS:
                for half, m in (("x", 0), ("w", NXC)):
                    wt = p_win.tile([128, DIM], bf16, name="win", tag="win")
                    nc.sync.dma_start(
                        wt[:].rearrange("p (kc m) -> p kc m", kc=NKD),
                        win_dram[mod][m])
                    tiles[(mod, half)] = wt
            return tiles

        def emit_A(blk, xTt, pre=None):
            gat = {}
            for mod in MODS:
                gat[mod] = p_gat.tile([128, NXC * LB], bf16, name="gat", tag="gat")
            for c in range(NXC):
                pp = {}
                for mod in MODS:
                    for half, m in (("x", c), ("w", NXC + c)):
                        if c == 0 and pre is not None:
                            wt = pre[(mod, half)]
                        else:
                            wt = p_win.tile([128, DIM], bf16, name="win", tag="win")
                            nc.sync.dma_start(
                                wt[:].rearrange("p (kc m) -> p kc m", kc=NKD),
                                win_dram[mod][m])
                        ps = p_ps.tile([128, LB], f32, name="ps", tag="ps")
                        for kc in range(NKD):
                            nc.tensor.matmul(
                                ps[:],
                                lhsT=wt[:, kc * 128:(kc + 1) * 128],
                                rhs=xTt[mod][:, kc * LB:(kc + 1) * LB],
                                start=(kc == 0), stop=(kc == NKD - 1))
                        pp[(mod, half)] = ps
                sv = {}
                for mod in MODS:
                    # silu straight out of PSUM, bf16 result (2x DVE gate rate)
                    s = p_sv.tile([128, LB], bf16, name="sv", tag="sv")
                    nc.scalar.activation(s[:], pp[(mod, "w")][:], AF.Silu)
                    sv[mod] = s
                for mi, (mod, other) in enumerate((("a", "v"), ("v", "a"))):
                    if conv_mode == "dve":
                        cm = "dve"
                    elif conv_mode == "gps":
                        cm = "gps"
                    elif conv_mode == "dve+gps":
                        cm = "dve" if mi == 0 else "gps"
                    else:
                        raise ValueError(conv_mode)
                    # bf16 conv: halves DVE MAC time; the tiny 3-col history
                    # prepend/save copies run on ACT, not DVE -- per-instr
                    # overhead on HW (~+124ns) makes 3-col DVE ops pure waste
                    axp = p_axp.tile([128, LB + 3], bf16, name="axp", tag="axp")
                    nc.scalar.copy(axp[:, 0:3],
                                   hist[mod][:, c * 3:(c + 1) * 3])
                    nc.scalar.copy(axp[:, 3:LB + 3], pp[(mod, "x")][:])
                    nc.scalar.copy(hist[mod][:, c * 3:(c + 1) * 3],
                                   axp[:, LB:LB + 3])
                    eng = nc.vector if cm == "dve" else nc.gpsimd
                    acc = p_sv.tile([128, LB], bf16, name="convacc", tag="convacc")
                    eng.tensor_scalar(
                        acc[:], axp[:, 0:LB],
                        cw_sb[mod][:, c * 4: c * 4 + 1],
                        cbc_sb[mod][:, c:c + 1], AOP.mult, AOP.add)
                    for t in range(1, 4):
                        acc2 = p_sv.tile([128, LB], bf16, name="convacc", tag="convacc")
                        eng.scalar_tensor_tensor(
                            acc2[:], axp[:, t:t + LB],
                            cw_sb[mod][:, c * 4 + t: c * 4 + t + 1],
                            acc[:], AOP.mult, AOP.add)
                        acc = acc2
                    nc.vector.tensor_mul(gat[mod][:, c * LB:(c + 1) * LB],
                                         acc[:], sv[other][:])

            return gat

        def emit_B(blk, gat, xts):
            l0 = blk * LB
            for mod in MODS:
                yoff = 0 if mod == "a" else L
                for n in range(NN):
                    po = [p_ps.tile([128, 512], f32, name="ps", tag="ps")
                          for _ in range(NLT)]
                    for c2 in range(NXC):
                        w = wout_sb[mod][:, (c2 * NN + n) * 512:
                                         (c2 * NN + n + 1) * 512]
                        for mt in range(NLT):
                            nc.tensor.matmul(
                                po[mt][:],
                                lhsT=gat[mod][:, c2 * LB + mt * 128:
                                              c2 * LB + (mt + 1) * 128],
                                rhs=w,
                                start=(c2 == 0), stop=(c2 == NXC - 1))
                    for mt in range(NLT):
                        yt = p_yout.tile([128, 512], bf16, name="yout", tag="yout")
                        nc.vector.tensor_add(
                            yt[:], po[mt][:],
                            xts[mod][mt][:, n * 512:(n + 1) * 512])
                        nc.sync.dma_start(
                            y[yoff + l0 + mt * 128: yoff + l0 + (mt + 1) * 128,
                              n * 512:(n + 1) * 512], yt[:])

        # Flattened (rep, blk) sequence. Per step: A(blk) is emitted, then
        # T(blk+1) and the next weight prefetch, then B(blk). The c0 weight
        # pair is prefetched before T(0) so the first matmul waits only on
        # the first xT transpose chain; conv state DMAs queue after x loads.
        nsteps = repeat * NB
        # Startup-ordered emission: mod-a's T chain first, then its c0 weight
        # tiles (so the first in_proj matmuls start as soon as xT_a lands),
        # then mod-v's chain + weights, and conv state (needed ~30us in) last.
        pre = {}

        def _prefetch_mod(mod):
            for half, m in (("x", 0), ("w", NXC)):
                wt = p_win.tile([128, DIM], bf16, name="win", tag="win")
                nc.sync.dma_start(
                    wt[:].rearrange("p (kc m) -> p kc m", kc=NKD),
                    win_dram[mod][m])
                pre[(mod, half)] = wt

        cur = emit_T(0, first=True, after_mod=_prefetch_mod)
        setup_conv_state()
        for step in range(nsteps):
            blk = step % NB
            gat = emit_A(blk, cur[0], pre)
            if step == 0:
                load_wout()
            if step + 1 < nsteps:
                pre = prefetch_w((step + 1) % NB)
                nxt = emit_T((step + 1) % NB)
            emit_B(blk, gat, cur[1])
            if step + 1 < nsteps:
                cur = nxt

    nc.finalize()
    return nc


def _get_nc(conv_mode=None, repeat=1):
    key = ("nc", conv_mode or CONV_MODE, repeat)
    if key not in _cache:
        _cache[key] = _build_nc(conv_mode, repeat)
    return _cache[key]


def _bf16(a):
    import ml_dtypes
    return np.asarray(a, np.float32).astype(ml_dtypes.bfloat16)


def _prep_weights(inputs):
    f = np.float32
    a_in = np.asarray(inputs["a_in_w"], f) * np.asarray(inputs["a_norm_w"], f)[None, :]
    v_in = np.asarray(inputs["v_in_w"], f) * np.asarray(inputs["v_norm_w"], f)[None, :]

    def pack_in(w):  # [2*INNER, DIM] -> [32, 128, 8, 128]: m-tile x [d%128, d//128, e]
        t = w.T.reshape(NKD, 128, 2 * NXC, 128)   # [kc, p, m, e]
        return _bf16(np.ascontiguousarray(t.transpose(2, 1, 0, 3)))

    def pack_out(w):  # [DIM, INNER] -> [128, NXC*NN*512]: [e%128, (e//128, d//512, d%512)]
        t = np.asarray(w, f).T.reshape(NXC, 128, NN, 512)   # [c2, p, n, d]
        return _bf16(np.ascontiguousarray(
            t.transpose(1, 0, 2, 3).reshape(128, NXC * NN * 512)))

    def pack_cw(w):  # [INNER, 1, 4] -> [128, 64]
        return np.ascontiguousarray(
            np.asarray(w, f)[:, 0, :].reshape(NXC, 128, 4)
            .transpose(1, 0, 2).reshape(128, NXC * 4))

    return {
        "wina": pack_in(a_in),
        "winv": pack_in(v_in),
        "wouta": pack_out(np.asarray(inputs["a_out_w"], f)),
        "woutv": pack_out(np.asarray(inputs["v_out_w"], f)),
        "cwa": pack_cw(inputs["a_conv_w"]),
        "cwv": pack_cw(inputs["v_conv_w"]),
        "cbca": np.ascontiguousarray(
            np.asarray(inputs["a_conv_b"], f).reshape(NXC, 128).T),
        "cbcv": np.ascontiguousarray(
            np.asarray(inputs["v_conv_b"], f).reshape(NXC, 128).T),
    }


def _make_in_maps(inputs):
    shared = _prep_weights(inputs)
    audio = _bf16(inputs["audio"])
    visual = _bf16(inputs["visual"])
    in_maps = []
    for b in range(B):
        m = dict(shared)
        m["xa"] = np.ascontiguousarray(audio[b])
        m["xv"] = np.ascontiguousarray(visual[b])
        in_maps.append(m)
    return in_maps


def _run(in_maps):
    from concourse.bass_utils import run_bass_kernel_spmd
    nc = _get_nc()
    out = None
    for _attempt in range(3):
        res = run_bass_kernel_spmd(nc, in_maps, core_ids=list(range(NCORES)))
        out = np.stack([res.results[b]["y"] for b in range(B)],
                       axis=0).astype(np.float32)
        # Rare terminal-side flake: a dropped write leaves a [128, 512] output
        # tile as the donated zero-buffer contents. Legit all-zero tiles are
        # impossible (residual stream is dense gaussian), so detect and retry.
        v = out.reshape(B, (2 * L) // 128, 128, DIM // 512, 512)
        if np.abs(v).max(axis=(2, 4)).min() > 0.0:
            return out
    return out


def kernel(**inputs) -> np.ndarray:
    return _run(_make_in_maps(inputs))



# revision 8
# speedup vs baseline: 1.0053x; 1.0053x over previous
"""AV temporal gated-conv MLP block for Trainium2 (8 NeuronCores, Bass/Tile).

Per-core strategy: pure data parallelism over the batch (B=8 -> 1 batch
element per core, both modalities on the same core since the gating couples
them). No collectives. Per core, loop over 4 l-blocks of 512 tokens:

  T: rms-norm in natural [l, d] layout -- ACT Square with accum_out for the
     square-sums, rsqrt as one batched [128,4] DVE Newton chain, per-partition
     scale, then DMA-XBAR transpose into the [128(d%128), 8(dc), 128(l)] xT
     slab (zero PE involvement).
  A: in_proj matmuls read SBUF-RESIDENT fp8-e3m4 weights (stationary fp8 x
     moving bf16 runs at full bf16 PE rate -- HW-verified exact); causal
     depthwise conv on VectorE in bf16; silu on ACT straight out of PSUM
     (with the 1/64 weight-scale folded into the activation input scale);
     cross-modal gate writes bf16 gat.
  B: out_proj matmuls with bf16 gat stationary x fp8-e3m4 resident wout
     moving; residual add folds the 1/128 wout scale.

ALL weights live in SBUF as fp8-e3m4 (4 mantissa bits), loaded ONCE at
startup in first-use order: win 8MB + wout 8MB -> 16MB resident. Weight
DMA per exec drops 72MB -> 16MB (and to ~0 steady-state per block), which
removes the PE/DMA fabric-contention tax (~1.17x measured on this setup)
that dominated the bf16-streaming baseline (817us slope-measured).

e3m4 quantization of the weights costs ~0.9% l2 rel err on top of the 0.37%
bf16-datapath baseline (sim-predicted composite ~1.2%, threshold 2e-2).
Scales: win x64 (silu gets scale=1/64; conv weights pre-divided by 64),
wout x128 (residual add multiplies PSUM by 1/128). All in-range, no clips.
"""
import sys

if "/opt/trn_rl_repo" not in sys.path:
    sys.path.insert(0, "/opt/trn_rl_repo")

import numpy as np

DIM = 1024
INNER = 2048
L = 2048
B = 8
NCORES = 8
EPS = 1e-5
LB = 512              # l-block (tokens per block)
NB = L // LB          # 4 blocks
NXC = INNER // 128    # 16 x-half e-chunks per modality
NKD = DIM // 128      # 8 contraction chunks for in_proj
NLT = LB // 128       # 4 l-tiles per block
NN = DIM // 512       # 2 out_proj n-tiles
WIN_SCALE = 64.0
WOUT_SCALE = 128.0

_cache = {}


def _build_nc(repeat=1):
    from contextlib import ExitStack

    import concourse.bass as bass
    import concourse.tile as tile
    from concourse import bacc, mybir

    dt = mybir.dt
    f32 = dt.float32
    bf16 = dt.bfloat16
    f8e3 = dt.float8e3
    i32 = dt.int32
    AOP = mybir.AluOpType
    AF = mybir.ActivationFunctionType

    nc = bacc.Bacc("TRN2", target_bir_lowering=False, debug=False,
                   num_devices=NCORES)

    x_dram = {
        "a": nc.dram_tensor("xa", [L, DIM], bf16, kind="ExternalInput").ap(),
        "v": nc.dram_tensor("xv", [L, DIM], bf16, kind="ExternalInput").ap(),
    }
    win_dram = {
        "a": nc.dram_tensor("wina", [2 * NXC, 128, NKD, 128], f8e3,
                            kind="ExternalInput").ap(),
        "v": nc.dram_tensor("winv", [2 * NXC, 128, NKD, 128], f8e3,
                            kind="ExternalInput").ap(),
    }
    wout_dram = {
        "a": nc.dram_tensor("wouta", [128, NXC * NN * 512], f8e3,
                            kind="ExternalInput").ap(),
        "v": nc.dram_tensor("woutv", [128, NXC * NN * 512], f8e3,
                            kind="ExternalInput").ap(),
    }
    cw_dram = {
        "a": nc.dram_tensor("cwa", [128, NXC * 4], f32, kind="ExternalInput").ap(),
        "v": nc.dram_tensor("cwv", [128, NXC * 4], f32, kind="ExternalInput").ap(),
    }
    cbc_dram = {
        "a": nc.dram_tensor("cbca", [128, NXC], f32, kind="ExternalInput").ap(),
        "v": nc.dram_tensor("cbcv", [128, NXC], f32, kind="ExternalInput").ap(),
    }
    y = nc.dram_tensor("y", [2 * L, DIM], bf16, kind="ExternalOutput").ap()

    MODS = ("a", "v")

    with tile.TileContext(nc) as tc, ExitStack() as ctx:
        sing = ctx.enter_context(tc.tile_pool(name="sing", bufs=1))
        p_xT = ctx.enter_context(tc.tile_pool(name="xT", bufs=2))
        p_gat = ctx.enter_context(tc.tile_pool(name="gat", bufs=2))
        p_xin = ctx.enter_context(tc.tile_pool(name="xin", bufs=14))
        p_xn = ctx.enter_context(tc.tile_pool(name="xn", bufs=6))
        p_stat = ctx.enter_context(tc.tile_pool(name="stat", bufs=4))
        p_axp = ctx.enter_context(tc.tile_pool(name="axp", bufs=4))
        p_sv = ctx.enter_context(tc.tile_pool(name="sv", bufs=3))
        p_yout = ctx.enter_context(tc.tile_pool(name="yout", bufs=4))
        p_ps = ctx.enter_context(
            tc.tile_pool(name="ps", bufs=8, space=bass.MemorySpace.PSUM))

        magic = sing.tile([128, NLT], i32, name="magic", tag="magic")
        nc.vector.memset(magic[:], 0x5F3759DF)

        cw_sb, cbc_sb, hist, win_sb, wout_sb = {}, {}, {}, {}, {}

        def setup_conv_state():
            for mod in MODS:
                cw_sb[mod] = sing.tile([128, NXC * 4], f32, name=f"cw_{mod}",
                                       tag=f"cw_{mod}")
                nc.scalar.dma_start(cw_sb[mod][:], cw_dram[mod][:])
                cbc_sb[mod] = sing.tile([128, NXC], f32, name=f"cbc_{mod}",
                                        tag=f"cbc_{mod}")
                nc.scalar.dma_start(cbc_sb[mod][:], cbc_dram[mod][:])
                hist[mod] = sing.tile([128, NXC * 3], bf16, name=f"hist_{mod}",
                                      tag=f"hist_{mod}")
                nc.vector.memset(hist[mod][:], 0.0)

        def load_win():
            # Resident fp8 in_proj weights: 16 chunked DMAs (= the HWDGE ring
            # depth, so the triggers never block the ACT sequencer), on the
            # Activation queue so the SP queue's x-loads/XBAR transposes can't
            # head-of-line block them. c-ordered: first 4 DMAs cover chunks
            # 0-3 of both halves/mods, so A(0) starts as soon as xT(0) lands.
            GM = 4  # m-tiles per DMA
            for mod in MODS:
                win_sb[mod] = sing.tile([128, 2 * NXC * NKD * 128], f8e3,
                                        name=f"win_{mod}", tag=f"win_{mod}")
                wout_sb[mod] = sing.tile([128, NXC * NN * 512], f8e3,
                                         name=f"wout_{mod}", tag=f"wout_{mod}")
            for g in range(NXC // GM):
                for mod in MODS:
                    for m0 in (g * GM, NXC + g * GM):
                        dst = win_sb[mod][:, m0 * NKD * 128:(m0 + GM) * NKD * 128]
                        nc.scalar.dma_start(
                            dst.rearrange("p (mm kc m) -> p mm kc m",
                                          mm=GM, kc=NKD),
                            win_dram[mod][m0:m0 + GM].rearrange(
                                "mm p kc m -> p mm kc m"))

        def load_wout():
            # out_proj weights on the SP queue (behind the T(0) transposes,
            # ahead of B(0)'s need by ~60us), 4 big DMAs per modality.
            NSP = 4
            csz = NXC * NN * 512 // NSP
            for s in range(NSP):
                for mod in MODS:
                    nc.sync.dma_start(wout_sb[mod][:, s * csz:(s + 1) * csz],
                                      wout_dram[mod][:, s * csz:(s + 1) * csz])

        def emit_T(blk, first=False):
            l0 = blk * LB
            xTt, xts = {}, {}
            # schedule the load+square chain ~a half block early (priority-only)
            for mod in MODS:
                boost = 0 if first else 600
                xTt[mod] = p_xT.tile([128, NKD * LB], bf16, name="xT", tag="xT")
                xts[mod] = []
                xns = []
                stats = p_stat.tile([128, NLT], f32, name="ssum", tag="ssum")
                with tc.high_priority(offset=boost):
                    for lt in range(NLT):
                        xt = p_xin.tile([128, DIM], bf16, name="xin", tag="xin")
                        nc.sync.dma_start(
                            xt[:], x_dram[mod][l0 + lt * 128: l0 + (lt + 1) * 128, :])
                        xnt = p_xn.tile([128, DIM], bf16, name="xn", tag="xn")
                        nc.scalar.activation(xnt[:], xt[:], AF.Square,
                                             accum_out=stats[:, lt:lt + 1])
                        xts[mod].append(xt)
                        xns.append(xnt)
                with tc.high_priority(offset=boost):
                    # rsqrt of all NLT square-sums in ONE batched [128, NLT]
                    # Newton chain (bit-trick seed + 3 iterations)
                    m = p_stat.tile([128, NLT], f32, name="mvar", tag="mvar")
                    nc.vector.tensor_scalar(m[:], stats[:], 1.0 / DIM, EPS,
                                            AOP.mult, AOP.add)
                    ts = p_stat.tile([128, NLT], f32, name="nsh", tag="nsh")
                    nc.vector.tensor_scalar(ts[:].bitcast(i32), m[:].bitcast(i32),
                                            1, None, AOP.logical_shift_right)
                    yv = p_stat.tile([128, NLT], f32, name="ny0", tag="ny0")
                    nc.vector.tensor_tensor(yv[:].bitcast(i32), magic[:],
                                            ts[:].bitcast(i32), AOP.subtract)
                    mh = p_stat.tile([128, NLT], f32, name="nmh", tag="nmh")
                    nc.vector.tensor_scalar(mh[:], m[:], -0.5, None, AOP.mult)
                    for it in range(3):
                        aa = p_stat.tile([128, NLT], f32, name="na", tag="na")
                        nc.vector.tensor_mul(aa[:], yv[:], yv[:])
                        cc = p_stat.tile([128, NLT], f32, name="ncc", tag="ncc")
                        nc.vector.tensor_tensor(cc[:], aa[:], mh[:], AOP.mult)
                        nc.vector.tensor_scalar(cc[:], cc[:], 1.0, 1.5,
                                                AOP.mult, AOP.add)
                        y2 = p_stat.tile([128, NLT], f32, name="nyi", tag="nyi")
                        nc.vector.tensor_mul(y2[:], yv[:], cc[:])
                        yv = y2
                    for lt in range(NLT):
                        nc.vector.tensor_scalar(xns[lt][:], xts[mod][lt][:],
                                                yv[:, lt:lt + 1], None, AOP.mult)
                with tc.high_priority(offset=boost):
                    for lt in range(NLT):
                        # DMA XBAR transpose: xn [128(l), 1024(d)] -> xT slab
                        dst = xTt[mod].rearrange("p (dc l) -> p dc l", dc=NKD)[
                            :, :, lt * 128:(lt + 1) * 128]
                        nc.sync.dma_start(dst, xns[lt][:], transpose=True)

            return xTt, xts

        def emit_A(blk, xTt):
            gat = {}
            for mod in MODS:
                gat[mod] = p_gat.tile([128, NXC * LB], bf16, name="gat", tag="gat")
            for c in range(NXC):
                pp = {}
                for mod in MODS:
                    for half, m in (("x", c), ("w", NXC + c)):
                        wt = win_sb[mod][:, m * NKD * 128:(m + 1) * NKD * 128]
                        ps = p_ps.tile([128, LB], f32, name="ps", tag="ps")
                        for kc in range(NKD):
                            nc.tensor.matmul(
                                ps[:],
                                lhsT=wt[:, kc * 128:(kc + 1) * 128],
                                rhs=xTt[mod][:, kc * LB:(kc + 1) * LB],
                                start=(kc == 0), stop=(kc == NKD - 1))
                        pp[(mod, half)] = ps
                sv = {}
                for mod in MODS:
                    # silu straight out of PSUM (1/WIN_SCALE folded into scale)
                    s = p_sv.tile([128, LB], bf16, name="sv", tag="sv")
                    nc.scalar.activation(s[:], pp[(mod, "w")][:], AF.Silu,
                                         scale=1.0 / WIN_SCALE)
                    sv[mod] = s
                for mod, other in (("a", "v"), ("v", "a")):
                    # bf16 conv on DVE; 3-col history prepend/save on ACT.
                    # axp holds WIN_SCALE-scaled x-half; cw is pre-divided.
                    axp = p_axp.tile([128, LB + 3], bf16, name="axp", tag="axp")
                    nc.scalar.copy(axp[:, 0:3],
                                   hist[mod][:, c * 3:(c + 1) * 3])
                    nc.scalar.copy(axp[:, 3:LB + 3], pp[(mod, "x")][:])
                    nc.scalar.copy(hist[mod][:, c * 3:(c + 1) * 3],
                                   axp[:, LB:LB + 3])
                    acc = p_sv.tile([128, LB], bf16, name="convacc", tag="convacc")
                    nc.vector.tensor_scalar(
                        acc[:], axp[:, 0:LB],
                        cw_sb[mod][:, c * 4: c * 4 + 1],
                        cbc_sb[mod][:, c:c + 1], AOP.mult, AOP.add)
                    for t in range(1, 4):
                        acc2 = p_sv.tile([128, LB], bf16, name="convacc", tag="convacc")
                        nc.vector.scalar_tensor_tensor(
                            acc2[:], axp[:, t:t + LB],
                            cw_sb[mod][:, c * 4 + t: c * 4 + t + 1],
                            acc[:], AOP.mult, AOP.add)
                        acc = acc2
                    nc.vector.tensor_mul(gat[mod][:, c * LB:(c + 1) * LB],
                                         acc[:], sv[other][:])

            return gat

        def emit_B(blk, gat, xts):
            l0 = blk * LB
            for mod in MODS:
                yoff = 0 if mod == "a" else L
                for n in range(NN):
                    po = [p_ps.tile([128, 512], f32, name="ps", tag="ps")
                          for _ in range(NLT)]
                    for c2 in range(NXC):
                        w = wout_sb[mod][:, (c2 * NN + n) * 512:
                                         (c2 * NN + n + 1) * 512]
                        for mt in range(NLT):
                            nc.tensor.matmul(
                                po[mt][:],
                                lhsT=gat[mod][:, c2 * LB + mt * 128:
                                              c2 * LB + (mt + 1) * 128],
                                rhs=w,
                                start=(c2 == 0), stop=(c2 == NXC - 1))
                    for mt in range(NLT):
                        yt = p_yout.tile([128, 512], bf16, name="yout", tag="yout")
                        # residual add with the 1/WOUT_SCALE fold
                        nc.vector.scalar_tensor_tensor(
                            yt[:], po[mt][:], 1.0 / WOUT_SCALE,
                            xts[mod][mt][:, n * 512:(n + 1) * 512],
                            AOP.mult, AOP.add)
                        nc.sync.dma_start(
                            y[yoff + l0 + mt * 128: yoff + l0 + (mt + 1) * 128,
                              n * 512:(n + 1) * 512], yt[:])

        nsteps = repeat * NB
        cur = emit_T(0, first=True)
        load_win()
        load_wout()
        setup_conv_state()
        for step in range(nsteps):
            blk = step % NB
            gat = emit_A(blk, cur[0])
            if step + 1 < nsteps:
                nxt = emit_T((step + 1) % NB)
            emit_B(blk, gat, cur[1])
            if step + 1 < nsteps:
                cur = nxt

    nc.finalize()
    return nc


def _get_nc(repeat=1):
    key = ("nc", repeat)
    if key not in _cache:
        _cache[key] = _build_nc(repeat)
    return _cache[key]


def _bf16(a):
    import ml_dtypes
    return np.asarray(a, np.float32).astype(ml_dtypes.bfloat16)


def _e3m4(a):
    import ml_dtypes
    return np.clip(np.asarray(a, np.float32), -15.5, 15.5).astype(
        ml_dtypes.float8_e3m4)


def _prep_weights(inputs):
    f = np.float32
    a_in = np.asarray(inputs["a_in_w"], f) * np.asarray(inputs["a_norm_w"], f)[None, :]
    v_in = np.asarray(inputs["v_in_w"], f) * np.asarray(inputs["v_norm_w"], f)[None, :]

    def pack_in(w):  # [2*INNER, DIM] -> [32, 128, 8, 128] e3m4 (x WIN_SCALE)
        t = (w * WIN_SCALE).T.reshape(NKD, 128, 2 * NXC, 128)   # [kc, p, m, e]
        return _e3m4(np.ascontiguousarray(t.transpose(2, 1, 0, 3)))

    def pack_out(w):  # [DIM, INNER] -> [128, NXC*NN*512] e3m4 (x WOUT_SCALE)
        t = (np.asarray(w, f) * WOUT_SCALE).T.reshape(NXC, 128, NN, 512)
        return _e3m4(np.ascontiguousarray(
            t.transpose(1, 0, 2, 3).reshape(128, NXC * NN * 512)))

    def pack_cw(w):  # [INNER, 1, 4] -> [128, 64], pre-divided by WIN_SCALE
        return np.ascontiguousarray(
            (np.asarray(w, f) / WIN_SCALE)[:, 0, :].reshape(NXC, 128, 4)
            .transpose(1, 0, 2).reshape(128, NXC * 4))

    return {
        "wina": pack_in(a_in),
        "winv": pack_in(v_in),
        "wouta": pack_out(np.asarray(inputs["a_out_w"], f)),
        "woutv": pack_out(np.asarray(inputs["v_out_w"], f)),
        "cwa": pack_cw(inputs["a_conv_w"]),
        "cwv": pack_cw(inputs["v_conv_w"]),
        "cbca": np.ascontiguousarray(
            np.asarray(inputs["a_conv_b"], f).reshape(NXC, 128).T),
        "cbcv": np.ascontiguousarray(
            np.asarray(inputs["v_conv_b"], f).reshape(NXC, 128).T),
    }


def _make_in_maps(inputs):
    shared = _prep_weights(inputs)
    audio = _bf16(inputs["audio"])
    visual = _bf16(inputs["visual"])
    in_maps = []
    for b in range(B):
        m = dict(shared)
        m["xa"] = np.ascontiguousarray(audio[b])
        m["xv"] = np.ascontiguousarray(visual[b])
        in_maps.append(m)
    return in_maps


def _run(in_maps):
    from concourse.bass_utils import run_bass_kernel_spmd
    nc = _get_nc()
    out = None
    for _attempt in range(3):
        res = run_bass_kernel_spmd(nc, in_maps, core_ids=list(range(NCORES)))
        out = np.stack([res.results[b]["y"] for b in range(B)],
                       axis=0).astype(np.float32)
        # Rare terminal-side flake: a dropped write leaves a [128, 512] output
        # tile as the donated zero-buffer contents. Legit all-zero tiles are
        # impossible (residual stream is dense gaussian), so detect and retry.
        v = out.reshape(B, (2 * L) // 128, 128, DIM // 512, 512)
        if np.abs(v).max(axis=(2, 4)).min() > 0.0:
            return out
    return out


def kernel(**inputs) -> np.ndarray:
    return _run(_make_in_maps(inputs))
